# revision 20
# baseline (speedup 1.0000x reference)
"""Trainium2 Bass kernel for a 2-layer pseudo-bidirectional LSTM (BiRNN).

Model (per reference):
  sfc1/sfc2 MLPs -> initial h/c for LSTM1
  LSTM1 over time-reversed input (T=60, H=64, in=9)
  LSTM2 over LSTM1 output (forward order), zero init
  out   = out2 @ Wout.T + bout          [B, T, 8]
  out_sfc = h2_last @ Wsfc_out.T + b    [B, 8]

Sharding: pure data parallelism, batch 8192 -> 8 cores x 1024.

Per-core layout: 2 pipelined "streams" of 512 batch; within a stream all
state/gate tiles are packed [128, 256]: partitions 0:64 = subchunk c0
(batch 0:256 of the stream), partitions 64:128 = subchunk c1.  Recurrent
state stays transposed ([hidden, batch]) so per-step matmuls need no
transposes.  All matmuls are plain full 128x128-mode ops with PSUM dst
base partition 0 (4-byte matmuls cannot target other partitions); the two
subchunks are handled in ONE matmul via block-diagonal weights
[[W,0],[0,W]].  Matmul inputs are float32r (full-rate PE, ~tf32
precision, fp32 PSUM accumulate).
"""

import numpy as np

import concourse.bass as bass
import concourse.bacc as bacc
import concourse.tile as tile
from concourse import mybir
from concourse.bass_utils import run_bass_kernel_spmd

F32 = mybir.dt.float32
F32R = mybir.dt.float32r
AF = mybir.ActivationFunctionType
OP = mybir.AluOpType

B_FULL = 8192
T = 60
NX = 9
NX_SFC = 17
H = 64
NY = 8
NY_SFC = 8
N_CORES = 8
B_CORE = B_FULL // N_CORES          # 1024
N_STREAMS = 2
BS = B_CORE // N_STREAMS            # 512 per stream
BC = BS // 2                        # 256 per subchunk

GATES = ("i", "f", "o", "g")        # psum region order; g last (tanh slice)
GATE_ROWS = {"i": 0, "f": 1, "g": 2, "o": 3}  # pytorch row order i,f,g,o


# ---------------------------------------------------------------------------
# weight packing (host side)
# ---------------------------------------------------------------------------

def _blockdiag(a):
    """[k, m] -> [128, 2m]: block at (0:k, 0:m) and (64:64+k, m:2m)."""
    k, m = a.shape
    assert k <= 64 and m <= 64
    out = np.zeros((128, 2 * m), np.float32)
    out[0:k, 0:m] = a
    out[64 : 64 + k, m : 2 * m] = a
    return out


class _WPack:
    def __init__(self):
        self.cols = []
        self.slices = {}
        self.arrs = []

    def add(self, name, arr):
        assert arr.shape[0] == 128, (name, arr.shape)
        off = sum(self.cols)
        self.cols.append(arr.shape[1])
        self.slices[name] = (off, arr.shape[1])
        self.arrs.append(arr.astype(np.float32))

    def pack(self):
        w = np.concatenate(self.arrs, axis=1)
        pad = (-w.shape[1]) % 4
        if pad:
            w = np.concatenate([w, np.zeros((128, pad), np.float32)], axis=1)
        return w


def build_weight_pack(inp):
    wp = _WPack()

    def gate_w(W4, g):
        r = GATE_ROWS[g] * H
        return np.asarray(W4[r : r + H], np.float32)

    bias1 = np.asarray(inp["bih1"], np.float32) + np.asarray(inp["bhh1"], np.float32)
    bias2 = np.asarray(inp["bih2"], np.float32) + np.asarray(inp["bhh2"], np.float32)

    def gate_b(bias4, g):
        r = GATE_ROWS[g] * H
        return bias4[r : r + H]

    for lname, Whh, Wih in (("l1", inp["Whh1"], inp["Wih1"]), ("l2", inp["Whh2"], inp["Wih2"])):
        for g in GATES:
            wp.add(f"{lname}_wh_{g}", _blockdiag(gate_w(Whh, g).T))
            wp.add(f"{lname}_wx_{g}", _blockdiag(np.asarray(gate_w(Wih, g).T)))
    # bank bias matmuls: lhsT [128, 128], rows 0/1 = bias patterns for the
    # bank's two gates (value bias_g[p % 64] at every partition p of M).
    for lname, bias4 in (("l1", bias1), ("l2", bias2)):
        for bank, (ga, gb) in enumerate(((GATES[0], GATES[1]), (GATES[2], GATES[3]))):
            bb = np.zeros((128, 128), np.float32)
            bb[0] = np.tile(gate_b(bias4, ga), 2)
            bb[1] = np.tile(gate_b(bias4, gb), 2)
            wp.add(f"{lname}_bb{bank}", bb)
    # indicator rhs [128, 512]: row0 -> cols 0:256, row1 -> cols 256:512
    ind = np.zeros((128, 512), np.float32)
    ind[0, 0:256] = 1.0
    ind[1, 256:512] = 1.0
    wp.add("ind", ind)
    # surface MLPs: [Wsfc.T; b] block-diagonal
    for nm, wkey, bkey in (("sfc1", "Wsfc1", "bsfc1"), ("sfc2", "Wsfc2", "bsfc2")):
        w = np.vstack(
            [np.asarray(inp[wkey], np.float32).T, np.asarray(inp[bkey], np.float32)[None, :]]
        )  # [18, 64]
        wp.add(nm, _blockdiag(w))
    # output heads: block-diag [128, 16]: out rows 0:8 = c0, 8:16 = c1
    wp.add("woutT", _blockdiag(np.asarray(inp["Wout"], np.float32).T))
    wp.add("wsfcoutT", _blockdiag(np.asarray(inp["Wsfc_out"], np.float32).T))
    # misc fp32 bias patterns for the DVE psum-evacuation adds
    yb = np.zeros((128, 1), np.float32)
    yb[0:NY, 0] = np.asarray(inp["bout"], np.float32)
    yb[NY : 2 * NY, 0] = np.asarray(inp["bout"], np.float32)
    wp.add("ybias", yb)
    sb = np.zeros((128, 1), np.float32)
    sb[0:NY_SFC, 0] = np.asarray(inp["bsfc_out"], np.float32)
    sb[NY_SFC : 2 * NY_SFC, 0] = np.asarray(inp["bsfc_out"], np.float32)
    wp.add("sfcbias", sb)

    return wp.pack(), wp.slices


# ---------------------------------------------------------------------------
# bass program
# ---------------------------------------------------------------------------

_CACHED = {}


def build_program(wcols):
    nc = bacc.Bacc("TRN2", target_bir_lowering=False, debug=False)
    io = dict(
        xaug=nc.dram_tensor("xaug", [T, N_STREAMS, 128, BC], F32, kind="ExternalInput"),
        aux=nc.dram_tensor("auxT", [N_STREAMS, 128, BC], F32, kind="ExternalInput"),
        wpack=nc.dram_tensor("wpack", [128, wcols], F32, kind="ExternalInput"),
        y_out=nc.dram_tensor("y_out", [T, NY, B_CORE], F32, kind="ExternalOutput"),
        sfc_out=nc.dram_tensor("sfc_out", [NY_SFC, B_CORE], F32, kind="ExternalOutput"),
        ys1=nc.dram_tensor("ys1", [T, N_STREAMS, 128, BC], F32R),
    )
    return nc, io


def emit_kernel(tc, io, wslices):
    nc = tc.nc
    xaug_d, aux_d, wpack_d = io["xaug"], io["aux"], io["wpack"]
    y_out_d, sfc_out_d, ys1_d = io["y_out"], io["sfc_out"], io["ys1"]

    import contextlib

    ctx = contextlib.ExitStack()
    consts = ctx.enter_context(tc.tile_pool(name="consts", bufs=1))
    statep = ctx.enter_context(tc.tile_pool(name="state", bufs=1))
    hpool = [ctx.enter_context(tc.tile_pool(name=f"h{s}", bufs=2)) for s in range(N_STREAMS)]
    xpool = [ctx.enter_context(tc.tile_pool(name=f"x{s}", bufs=3)) for s in range(N_STREAMS)]
    sigp = [ctx.enter_context(tc.tile_pool(name=f"sig{s}", bufs=2)) for s in range(N_STREAMS)]
    tmpp = [ctx.enter_context(tc.tile_pool(name=f"tmp{s}", bufs=2)) for s in range(N_STREAMS)]
    ysbp = [ctx.enter_context(tc.tile_pool(name=f"ysb{s}", bufs=2)) for s in range(N_STREAMS)]
    gpsum = [
        ctx.enter_context(tc.tile_pool(name=f"gp{s}", bufs=1, space="PSUM"))
        for s in range(N_STREAMS)
    ]
    ypsum = [
        ctx.enter_context(tc.tile_pool(name=f"yp{s}", bufs=2, space="PSUM"))
        for s in range(N_STREAMS)
    ]

    # ---- constants (weights live in SBUF as f32r; misc patterns read as f32) ----
    wtile = consts.tile([128, wpack_d.shape[1]], F32R, tag="wpack", name="wtile")
    nc.sync.dma_start(wtile[:], wpack_d[:].bitcast(F32R))

    def wsl(name):
        off, w = wslices[name]
        return wtile[:, off : off + w]

    def wsl32(name):
        return wsl(name).bitcast(F32)

    aux_t = consts.tile([128, N_STREAMS * BC], F32R, tag="aux", name="aux_t")
    for s in range(N_STREAMS):
        nc.sync.dma_start(aux_t[:, s * BC : (s + 1) * BC], aux_d[s].bitcast(F32R))

    C1 = [statep.tile([128, BC], F32, tag=f"C1_{s}", name=f"C1_{s}") for s in range(N_STREAMS)]
    C2 = [statep.tile([128, BC], F32, tag=f"C2_{s}", name=f"C2_{s}") for s in range(N_STREAMS)]

    def mm(out, lhsT, rhs, start, stop):
        nc.tensor.matmul(out, lhsT, rhs, start=start, stop=stop, skip_group_check=True)

    # ---- surface MLPs -> H1_0, C1 ----
    H1 = [None] * N_STREAMS
    for s in range(N_STREAMS):
        hps = gpsum[s].tile([128, BC], F32, tag="gates", name="hps")
        cps = gpsum[s].tile([128, BC], F32, tag="gates", name="cps")
        mm(hps[:], wsl("sfc1"), aux_t[:, s * BC : (s + 1) * BC], True, True)
        mm(cps[:], wsl("sfc2"), aux_t[:, s * BC : (s + 1) * BC], True, True)
        H1[s] = hpool[s].tile([128, BC], F32R, tag="H", name="H1t")
        nc.scalar.activation(H1[s][:], hps[:], AF.Tanh)
        nc.scalar.activation(C1[s][:], cps[:], AF.Tanh)

    # ---- LSTM step ----
    def lstm_step(layer, t, s, H_prev, yjobs):
        ps = gpsum[s].tile([128, 4 * BC], F32, tag="gates", name="gates_ps")
        xa = xpool[s].tile([128, BC], F32R, tag="xa", name="xa")
        if layer == 1:
            nc.sync.dma_start(xa[:], xaug_d[t, s].bitcast(F32R))
        else:
            nc.sync.dma_start(xa[:], ys1_d[T - 1 - t, s])

        ind = wsl("ind")
        ln = f"l{layer}"
        # bank bias matmuls open each bank's accumulation group (M=128 covers
        # all partitions; start=True pending-zeroes the whole 2KB bank).
        for bank in range(2):
            bsl = ps[:, bank * 2 * BC : (bank + 1) * 2 * BC]
            mm(bsl, wsl(f"{ln}_bb{bank}"), ind, True, False)
        for gi, g in enumerate(GATES):
            reg = ps[:, gi * BC : (gi + 1) * BC]
            mm(reg, wsl(f"{ln}_wh_{g}"), H_prev[:], False, False)
            mm(reg, wsl(f"{ln}_wx_{g}"), xa[:], False, False)

        C = C1[s] if layer == 1 else C2[s]
        sig = sigp[s].tile([128, 3 * BC], F32, tag="sig", name="sig")
        nc.scalar.activation(sig[:], ps[:, 0 : 3 * BC], AF.Sigmoid)
        tg = tmpp[s].tile([128, BC], F32, tag="tg", name="tg")
        nc.scalar.activation(tg[:], ps[:, 3 * BC : 4 * BC], AF.Tanh)
        p = tmpp[s].tile([128, BC], F32, tag="p", name="p")
        nc.vector.tensor_tensor(p[:], sig[:, 0:BC], tg[:], OP.mult)          # i*g
        nc.vector.tensor_tensor(C[:], C[:], sig[:, BC : 2 * BC], OP.mult)    # c *= f
        nc.vector.tensor_tensor(C[:], C[:], p[:], OP.add)                    # c += i*g
        tc_t = tmpp[s].tile([128, BC], F32, tag="tc", name="tct")
        nc.scalar.activation(tc_t[:], C[:], AF.Tanh)
        Hn = hpool[s].tile([128, BC], F32R, tag="H", name="Hn")
        nc.vector.tensor_tensor(Hn[:], sig[:, 2 * BC : 3 * BC], tc_t[:], OP.mult)

        if layer == 1:
            nc.sync.dma_start(ys1_d[t, s], Hn[:])
        else:
            # y head: one block-diag matmul -> [16, BC]; 2 steps share a
            # [16, 2*BC] psum bank in the free dim.
            if t % 2 == 0:
                yjobs[s] = ypsum[s].tile([16, 2 * BC], F32, tag="ypy", name="yps")
            yps = yjobs[s]
            c0 = (t % 2) * BC
            mm(yps[:, c0 : c0 + BC], wsl("woutT"), Hn[:], t % 2 == 0, t % 2 == 1)
            if t % 2 == 1:
                ysb = ysbp[s].tile([16, 2 * BC], F32, tag="ysb", name="ysb")
                nc.vector.tensor_scalar(ysb[:], yps[:], wsl32("ybias")[0:16, :], None, OP.add)
                for k in range(2):
                    te = t - 1 + k
                    nc.sync.dma_start(
                        y_out_d[te, :, s * BS : s * BS + BC], ysb[0:NY, k * BC : (k + 1) * BC]
                    )
                    nc.sync.dma_start(
                        y_out_d[te, :, s * BS + BC : (s + 1) * BS],
                        ysb[NY : 2 * NY, k * BC : (k + 1) * BC],
                    )
        return Hn

    yjobs = [None] * N_STREAMS
    for t in range(T):
        for s in range(N_STREAMS):
            H1[s] = lstm_step(1, t, s, H1[s], yjobs)

    H2 = [None] * N_STREAMS
    for s in range(N_STREAMS):
        H2[s] = hpool[s].tile([128, BC], F32R, tag="H", name="H2t")
        nc.vector.memset(H2[s][:].bitcast(F32), 0.0)
        nc.vector.memset(C2[s][:], 0.0)

    for t in range(T):
        for s in range(N_STREAMS):
            H2[s] = lstm_step(2, t, s, H2[s], yjobs)

    # final surface output from last h2
    for s in range(N_STREAMS):
        sps = ypsum[s].tile([16, BC], F32, tag="ypy", name="sps")
        mm(sps[:], wsl("wsfcoutT"), H2[s][:], True, True)
        ssb = ysbp[s].tile([16, BC], F32, tag="ysb", name="ssb")
        nc.vector.tensor_scalar(ssb[:], sps[:], wsl32("sfcbias")[0:16, :], None, OP.add)
        nc.sync.dma_start(sfc_out_d[:, s * BS : s * BS + BC], ssb[0:NY_SFC, :])
        nc.sync.dma_start(
            sfc_out_d[:, s * BS + BC : (s + 1) * BS], ssb[NY_SFC : 2 * NY_SFC, :]
        )

    ctx.close()


# ---------------------------------------------------------------------------
# host entry
# ---------------------------------------------------------------------------

def prep_inputs(inputs):
    wpack, wslices = build_weight_pack(inputs)

    xm = np.asarray(inputs["inputs_main"], np.float32)     # [B, T, 9]
    ax = np.asarray(inputs["inputs_aux"], np.float32)      # [B, 17]
    xf = xm[:, ::-1, :]                                    # reverse time

    # xaug [T, ncore? streams...] packed per core below
    xT = np.transpose(xf, (1, 2, 0))                       # [T, 9, B]
    axT = ax.T                                             # [17, B]

    in_maps = []
    for c in range(N_CORES):
        bsl = slice(c * B_CORE, (c + 1) * B_CORE)
        xTc = xT[:, :, bsl]                                # [T, 9, 1024]
        axc = axT[:, bsl]                                  # [17, 1024]
        xaug = np.zeros((T, N_STREAMS, 128, BC), np.float32)
        aux = np.zeros((N_STREAMS, 128, BC), np.float32)
        for s in range(N_STREAMS):
            for cc in range(2):
                cols = slice(s * BS + cc * BC, s * BS + (cc + 1) * BC)
                r0 = cc * 64
                xaug[:, s, r0 : r0 + NX, :] = xTc[:, :, cols]
                aux[s, r0 : r0 + NX_SFC, :] = axc[:, cols]
                aux[s, r0 + NX_SFC, :] = 1.0
        in_maps.append(
            {"xaug": xaug, "auxT": aux, "wpack": wpack}
        )
    return in_maps, wpack.shape[1], wslices


def get_program(wcols, wslices):
    key = (wcols,)
    if key in _CACHED:
        return _CACHED[key]
    nc, io = build_program(wcols)
    with tile.TileContext(nc) as tc:
        emit_kernel(tc, io, wslices)
    nc.compile()
    _CACHED[key] = nc
    return nc


def gather_outputs(results):
    outs, sfcs = [], []
    for r in results:
        outs.append(np.transpose(r["y_out"], (2, 0, 1)))   # [B_CORE, T, NY]
        sfcs.append(r["sfc_out"].T)                        # [B_CORE, NY_SFC]
    return (
        np.concatenate(outs, axis=0).astype(np.float32),
        np.concatenate(sfcs, axis=0).astype(np.float32),
    )


def kernel(**inputs):
    in_maps, wcols, wslices = prep_inputs(inputs)
    nc = get_program(wcols, wslices)
    res = run_bass_kernel_spmd(nc, in_maps, list(range(N_CORES)))
    return gather_outputs(res.results)
